# revision 25
# baseline (speedup 1.0000x reference)
"""MPNCOV (iSQRT-COV pooling) Trainium2 kernel.

Math per sample (C=256 channels, M=196 spatial):
  xc   = x - mean_m(x)                      # center along spatial dim
  A    = xc @ xc^T / sum(xc^2)              # = cov / trace(cov)
  Newton-Schulz (ITER_N=3) on A, final y = sqrt(normA) * YZY, triu-packed.

Scale folding: every intermediate X is stored as X_s with X = sigma_X * X_s,
sigma tracked symbolically so each PSUM->SBUF transform is a single
tensor_tensor subtract against a constant diagonal tile:
  ZY1_s = 3I   - A_s          (sigma 1/2)
  Y1_s  = A_s @ ZY1_s         (sigma 1/2)
  W1_s  = ZY1_s @ Y1_s        (sigma 1/4)
  ZY2_s = 12I  - W1_s         (sigma 1/8)
  Y2_s  = Y1_s @ ZY2_s        (sigma 1/16)
  Z2_s  = ZY2_s @ ZY1_s       (sigma 1/16)
  W2_s  = Z2_s @ Y2_s         (sigma 1/256)
  ZY3_s = 768I - W2_s
  F_s   = Y2_s @ ZY3_s,   y = (sqrt(tr/M)/8192) * F_s
All intermediates are polynomials in symmetric A => symmetric, so row-tiles
serve directly as matmul lhsT (no transposes in the NS chain). The only PE
transposes build xc^T for the Gram matmul; 1/sqrt(sum xc^2) is folded into
the transpose's PSUM->SBUF copy so the Gram directly yields A_s.

Matrices are stored as single [128, 512] tiles: cols 0:256 = matrix rows
0:128, cols 256:512 = matrix rows 128:256. Each product lands in ONE fp32
PSUM bank [128, 512] (two N=256 matmul groups), so every PSUM->SBUF
transform is one 512-wide DVE/ACT op. Matmul inputs are fp16 (1 cyc/row on
the PE + fast weight load); PSUM accumulation stays fp32.

Sharding: pure data parallel, batch 256 -> 32 samples on each of 8 cores.
Triu packing: all 32 per-sample results stay SBUF-resident; at the end one
DMA per matrix row r moves that row's triu tail for all 32 samples
(constant strides in both src and dst), alternating sync/scalar HWDGE.
"""

import numpy as np

from concourse import bacc, bass, bass_isa, mybir, tile
from concourse import bass_utils

F32 = mybir.dt.float32
P = 128
C = 256
M = 196
B = 256
NCORES = 8
S = B // NCORES            # samples per core
NTRIU = C * (C + 1) // 2   # 32896

# matmul input dtype for the big products
MM_DT = mybir.dt.float16

LAST_EXEC_NS = None
LAST_RESULTS = None


def build(tc, y_ap, x_ap, ident_ap, icons_ap, n_samples=S):
    nc = tc.nc
    import contextlib

    with contextlib.ExitStack() as ctx:
        consts = ctx.enter_context(tc.tile_pool(name="consts", bufs=1))
        fpool = ctx.enter_context(tc.tile_pool(name="fpool", bufs=1))
        work = ctx.enter_context(tc.tile_pool(name="work", bufs=2))
        mats = ctx.enter_context(tc.tile_pool(name="mats", bufs=2))
        psum = ctx.enter_context(tc.tile_pool(name="psum", bufs=8, space="PSUM"))

        ident = consts.tile([P, P], MM_DT, tag="ident")
        nc.sync.dma_start(ident[:], ident_ap[:])
        icons = consts.tile([P, 3, 2 * C], MM_DT, tag="icons")
        nc.sync.dma_start(icons[:], icons_ap[:])

        # Per 8-sample group: [P, 8, 384] — cols 0:256 = F rows 0:128 (full),
        # cols 256:384 = F22 (rows 128:256, cols 128:256). F21 = F12^T on host.
        GRP = 8
        ngrp = (n_samples + GRP - 1) // GRP
        ftg = [
            fpool.tile([P, GRP, 384], F32, tag=f"ft{g}", name=f"ft{g}")
            for g in range(ngrp)
        ]
        flushed = [False] * ngrp
        _dbg_x = None

        def prod(U, V):
            """One [128,512] PSUM bank <- U @ V (both [P,512] fp16, symmetric)."""
            p_t = psum.tile([P, 2 * C], F32, tag="ps_big")
            for mt in range(2):
                oc = slice(mt * C, (mt + 1) * C)
                ms0 = slice(mt * P, mt * P + P)
                ms1 = slice(C + mt * P, C + mt * P + P)
                nc.tensor.matmul(
                    p_t[:, oc], U[:, ms0], V[:, 0:C], start=True, stop=False
                )
                nc.tensor.matmul(
                    p_t[:, oc], U[:, ms1], V[:, C : 2 * C], start=False, stop=True
                )
            return p_t

        def prod_f(U, V):
            """Final product: skip F21 (host reconstructs it from F12^T).
            cols 0:256 = F rows 0:128 full; cols 384:512 = F22."""
            p_t = psum.tile([P, 2 * C], F32, tag="ps_big")
            nc.tensor.matmul(p_t[:, 0:C], U[:, 0:P], V[:, 0:C], start=True, stop=False)
            nc.tensor.matmul(
                p_t[:, 0:C], U[:, C : C + P], V[:, C : 2 * C], start=False, stop=True
            )
            nc.tensor.matmul(
                p_t[:, C + P : 2 * C], U[:, P:C], V[:, P:C], start=True, stop=False
            )
            nc.tensor.matmul(
                p_t[:, C + P : 2 * C], U[:, C + P : 2 * C], V[:, C + P : 2 * C],
                start=False, stop=True,
            )
            return p_t

        def sample_stages(b):
            """Yield closures for one sample's pipeline stages; tiles tagged
            by b%2 so a pair of samples uses disjoint pool slots and their
            PE bursts interleave (keeps the PE dense enough to stay warm)."""
            x = {}
            nonlocal _dbg_x
            _dbg_x = x
            fx = f"_{b % 3}"

            def load():
                x["xr"] = work.tile([P, 2, M], F32, tag="xr" + fx, name="xr" + fx)
                nc.sync.dma_start(
                    x["xr"][:], x_ap[b].rearrange("(h p) m -> p h m", p=P)
                )

            def stats():
                xr = x["xr"]
                mean2 = work.tile([P, 2], F32, tag="mean2" + fx, name="mean2" + fx)
                nc.vector.tensor_reduce(
                    mean2[:], xr[:], axis=mybir.AxisListType.X,
                    op=mybir.AluOpType.add,
                )
                negmean = work.tile([P, 2], F32, tag="negmean" + fx, name="nm" + fx)
                nc.vector.tensor_scalar_mul(negmean[:], mean2[:], -1.0 / M)
                xc = work.tile([P, 2, M], MM_DT, tag="xc" + fx, name="xc" + fx)
                for h in range(2):
                    nc.gpsimd.tensor_scalar_add(
                        xc[:, h], xr[:, h], negmean[:, h : h + 1]
                    )
                # sum(xc^2): square on DVE (fp16 2x rate), reduce per half
                sq = work.tile([P, 2, M], MM_DT, tag="sq" + fx, name="sq" + fx)
                nc.vector.tensor_mul(sq[:], xc[:], xc[:])
                s2 = work.tile([P, 2], F32, tag="s2" + fx, name="s2" + fx)
                nc.vector.tensor_reduce(
                    s2[:], sq[:], axis=mybir.AxisListType.X,
                    op=mybir.AluOpType.add,
                )
                x["xc"], x["s2"] = xc, s2

            def trace():
                s2r = work.tile([P, 2], F32, tag="s2r" + fx, name="s2r" + fx)
                nc.gpsimd.partition_all_reduce(
                    s2r[:], x["s2"][:], channels=P, reduce_op=bass_isa.ReduceOp.add
                )
                trv = work.tile([P, 1], F32, tag="trv" + fx, name="trv" + fx)
                nc.vector.tensor_tensor(
                    trv[:], s2r[:, 0:1], s2r[:, 1:2], op=mybir.AluOpType.add
                )
                inv = work.tile([P, 1], F32, tag="inv" + fx, name="inv" + fx)
                abv2 = work.tile([P, 1], F32, tag="abv2" + fx, name="abv2" + fx)
                nc.vector.reciprocal(inv[:], trv[:])
                nc.scalar.activation(
                    abv2[:], trv[:], mybir.ActivationFunctionType.Sqrt,
                    scale=1.0 / (M * 8192.0 * 8192.0),
                )
                x["inv"], x["abv2"] = inv, abv2

            def transpose():
                xc = x["xc"]
                tp = psum.tile([P, 2 * C], MM_DT, tag="ps_big", name="tp" + fx)
                for h in range(2):
                    nc.tensor.transpose(
                        tp[:, h * P : h * P + P], xc[:, h, 0:P], ident[:]
                    )
                    nc.tensor.transpose(
                        tp[0 : M - P, C + h * P : C + h * P + P], xc[:, h, P:M],
                        ident[:],
                    )
                x["tp"] = tp

            def scale_xcT():
                tp = x["tp"]
                xcT0 = work.tile([P, C], MM_DT, tag="xcT0" + fx, name="xcT0" + fx)
                xcT1 = work.tile([P, C], MM_DT, tag="xcT1" + fx, name="xcT1" + fx)
                nc.scalar.activation(
                    xcT0[:], tp[:, 0:C], mybir.ActivationFunctionType.Copy
                )
                nc.vector.tensor_copy(xcT1[0 : M - P], tp[0 : M - P, C : 2 * C])
                x["xcT0"], x["xcT1"] = xcT0, xcT1

            def gram():
                xcT0, xcT1 = x["xcT0"], x["xcT1"]
                a_ps = psum.tile([P, 2 * C], F32, tag="ps_big", name="aps" + fx)
                for mt in range(2):
                    oc = slice(mt * C, (mt + 1) * C)
                    ms = slice(mt * P, (mt + 1) * P)
                    nc.tensor.matmul(
                        a_ps[:, oc], xcT0[:, ms], xcT0[:], start=True, stop=False
                    )
                    nc.tensor.matmul(
                        a_ps[:, oc], xcT1[0 : M - P, ms], xcT1[0 : M - P, :],
                        start=False, stop=True,
                    )
                x["a_ps"] = a_ps

            def mat(tag):
                t = mats.tile([P, 2 * C], MM_DT, tag=tag + fx, name=tag + fx)
                x[tag] = t
                return t

            def drain_A():
                # A_s = (M*cov)/trace(M*cov): fold 1/tr into the PSUM drain
                nc.scalar.activation(
                    mat("A")[:], x["a_ps"][:], mybir.ActivationFunctionType.Copy,
                    scale=x["inv"][:, 0:1],
                )

            def zy1():
                nc.gpsimd.tensor_tensor(
                    mat("ZY1")[:], icons[:, 0, :], x["A"][:],
                    op=mybir.AluOpType.subtract,
                )

            def mk_prod(dst, u, v):
                def f():
                    x[dst] = prod(x[u], x[v])
                return f

            def mk_prod_f():
                x["f_ps"] = prod_f(x["Y2"], x["ZY3"])

            def drain(dst, src, eng):
                def f():
                    t = mat(dst)
                    if eng == "act":
                        nc.scalar.activation(
                            t[:], x[src][:], mybir.ActivationFunctionType.Copy
                        )
                    else:
                        nc.vector.tensor_copy(t[:], x[src][:])
                return f

            def sub(dst, k, src):
                def f():
                    nc.vector.tensor_tensor(
                        mat(dst)[:], icons[:, k, :], x[src][:],
                        op=mybir.AluOpType.subtract,
                    )
                return f

            def fstore():
                f_ps, abv2 = x["f_ps"], x["abv2"]
                ft = ftg[b // GRP]
                bi = b % GRP
                nc.vector.tensor_scalar_mul(
                    ft[:, bi, 0:C], f_ps[:, 0:C], abv2[:, 0:1]
                )
                nc.scalar.activation(
                    ft[:, bi, C : C + P], f_ps[:, C + P : 2 * C],
                    mybir.ActivationFunctionType.Copy, scale=abv2[:, 0:1],
                )

            return [
                load, stats, trace, transpose, scale_xcT, gram,
                drain_A, zy1,
                mk_prod("y1_ps", "A", "ZY1"), drain("Y1", "y1_ps", "act"),
                mk_prod("w1_ps", "ZY1", "Y1"), sub("ZY2", 1, "w1_ps"),
                mk_prod("y2_ps", "Y1", "ZY2"), drain("Y2", "y2_ps", "act"),
                mk_prod("z2_ps", "ZY2", "ZY1"), drain("Z2", "z2_ps", "act"),
                mk_prod("w2_ps", "Z2", "Y2"), sub("ZY3", 2, "w2_ps"),
                mk_prod_f, fstore,
            ]

        if n_samples == 1:
            sstg = sample_stages(0)
            for st in sstg:
                st()
            dbg = {}
            for nm_ in ["A", "ZY1", "Y1", "ZY2", "Y2", "Z2", "ZY3"]:
                d_ap = nc.dram_tensor(f"dbg_{nm_}", (P, 2 * C), F32,
                                      kind="ExternalOutput").ap()
                t32 = work.tile([P, 2 * C], F32, tag=f"dbg{nm_}")
                nc.vector.tensor_copy(t32[:], _dbg_x[nm_][:])
                nc.sync.dma_start(d_ap[:], t32[:])
            for nm_ in ["inv", "abv2", "s2"]:
                d_ap = nc.dram_tensor(f"dbg_{nm_}", (P, 1), F32,
                                      kind="ExternalOutput").ap()
                nc.sync.dma_start(d_ap[:], _dbg_x[nm_][:, 0:1])
            for nm_, rr in [("xcT0", P), ("xcT1", M - P)]:
                d_ap = nc.dram_tensor(f"dbg_{nm_}", (P, C), F32,
                                      kind="ExternalOutput").ap()
                t32 = work.tile([P, C], F32, tag=f"dbg{nm_}")
                nc.vector.tensor_copy(t32[0:rr], _dbg_x[nm_][0:rr])
                nc.sync.dma_start(d_ap[0:rr], t32[0:rr])
            return

        for b0 in range(0, n_samples, 3):
            grp = [sample_stages(b) for b in range(b0, min(b0 + 3, n_samples))]
            n = len(grp[0])
            for step in range(n + 2):
                for i, sg in enumerate(grp):
                    if 0 <= step - i < n:
                        sg[step - i]()
            # flush any output group fully produced by now (overlaps compute)
            done = min(b0 + 3, n_samples)
            for g in range(ngrp):
                if g * GRP + GRP <= done and not flushed[g]:
                    nc.sync.dma_start(
                        y_ap[g * GRP : (g + 1) * GRP].rearrange("s p c -> p s c"),
                        ftg[g][:],
                    )
                    flushed[g] = True
        for g in range(ngrp):  # tail flush (partial last group)
            if not flushed[g]:
                w = min(n_samples - g * GRP, GRP)
                nc.sync.dma_start(
                    y_ap[g * GRP : g * GRP + w].rearrange("s p c -> p s c"),
                    ftg[g][:, 0:w],
                )
                flushed[g] = True


def _make_const_inputs():
    # icons[:, k, :]: [3I, 12I, 768I] in concatenated row-tile layout:
    # cols 0:256 = matrix rows 0:128 (diag at col p),
    # cols 256:512 = matrix rows 128:256 (diag at col 256+128+p).
    e = np.zeros((P, 2 * C), np.float32)
    e[np.arange(P), np.arange(P)] = 1.0
    e[np.arange(P), C + P + np.arange(P)] = 1.0
    icons = np.stack([3.0 * e, 12.0 * e, 768.0 * e], axis=1).astype(np.float16)
    return {
        "ident": np.eye(P, dtype=np.float16),
        "icons": np.ascontiguousarray(icons),
    }


def make_nc(n_samples=S, num_devices=NCORES):
    nc = bacc.Bacc(
        "TRN2",
        target_bir_lowering=False,
        debug=False,
        enable_asserts=False,
        num_devices=num_devices,
    )
    x_ap = nc.dram_tensor("x", (n_samples, C, M), F32, kind="ExternalInput").ap()
    y_ap = nc.dram_tensor("y", (n_samples, P, 384), F32, kind="ExternalOutput").ap()
    ident_ap = nc.dram_tensor("ident", (P, P), MM_DT, kind="ExternalInput").ap()
    icons_ap = nc.dram_tensor("icons", (P, 3, 2 * C), MM_DT, kind="ExternalInput").ap()
    with tile.TileContext(nc) as tc:
        build(tc, y_ap, x_ap, ident_ap, icons_ap, n_samples)
    nc.compile()
    return nc


def kernel(x, _trace=False, **_trace_kwargs):
    global LAST_EXEC_NS, LAST_RESULTS
    x = np.ascontiguousarray(np.asarray(x), dtype=np.float32)
    assert x.shape == (B, C, 14, 14)
    xr = x.reshape(B, C, M)

    nc = make_nc()
    consts = _make_const_inputs()
    in_maps = [
        {"x": np.ascontiguousarray(xr[i * S : (i + 1) * S]), **consts}
        for i in range(NCORES)
    ]
    res = bass_utils.run_bass_kernel_spmd(
        nc, in_maps, core_ids=list(range(NCORES)), trace=_trace, **_trace_kwargs
    )
    LAST_EXEC_NS = res.exec_time_ns
    LAST_RESULTS = res
    yd = np.concatenate([r["y"] for r in res.results], axis=0)  # [B, 128, 384]
    full = np.empty((B, C, C), np.float32)
    full[:, 0:P, :] = yd[:, :, 0:C]                       # F rows 0:128
    full[:, P:C, P:C] = yd[:, :, C : C + P]               # F22
    full[:, P:C, 0:P] = yd[:, :, P:C].transpose(0, 2, 1)  # F21 = F12^T
    i, j = np.triu_indices(C)
    return np.ascontiguousarray(full.reshape(B, C * C)[:, i * C + j])



# revision 27
# speedup vs baseline: 3.7823x; 3.7823x over previous
"""MPNCOV (iSQRT-COV pooling) Trainium2 kernel.

Math per sample (C=256 channels, M=196 spatial):
  xc   = x - mean_m(x)                      # center along spatial dim
  A    = xc @ xc^T / sum(xc^2)              # = cov / trace(cov)
  Newton-Schulz (ITER_N=3) on A, final y = sqrt(normA) * YZY, triu-packed.

Scale folding: every intermediate X is stored as X_s with X = sigma_X * X_s,
sigma tracked symbolically so each PSUM->SBUF transform is a single
tensor_tensor subtract against a constant diagonal tile:
  ZY1_s = 3I   - A_s          (sigma 1/2)
  Y1_s  = A_s @ ZY1_s         (sigma 1/2)
  W1_s  = ZY1_s @ Y1_s        (sigma 1/4)
  ZY2_s = 12I  - W1_s         (sigma 1/8)
  Y2_s  = Y1_s @ ZY2_s        (sigma 1/16)
  Z2_s  = ZY2_s @ ZY1_s       (sigma 1/16)
  W2_s  = Z2_s @ Y2_s         (sigma 1/256)
  ZY3_s = 768I - W2_s
  F_s   = Y2_s @ ZY3_s,   y = (sqrt(tr/M)/8192) * F_s
All intermediates are polynomials in symmetric A => symmetric, so row-tiles
serve directly as matmul lhsT (no transposes in the NS chain). The only PE
transposes build xc^T for the Gram matmul; 1/sqrt(sum xc^2) is folded into
the transpose's PSUM->SBUF copy so the Gram directly yields A_s.

Matrices are stored as single [128, 512] tiles: cols 0:256 = matrix rows
0:128, cols 256:512 = matrix rows 128:256. Each product lands in ONE fp32
PSUM bank [128, 512] (two N=256 matmul groups), so every PSUM->SBUF
transform is one 512-wide DVE/ACT op. Matmul inputs are fp16 (1 cyc/row on
the PE + fast weight load); PSUM accumulation stays fp32.

Sharding: pure data parallel, batch 256 -> 32 samples on each of 8 cores.
Triu packing: all 32 per-sample results stay SBUF-resident; at the end one
DMA per matrix row r moves that row's triu tail for all 32 samples
(constant strides in both src and dst), alternating sync/scalar HWDGE.
"""

import numpy as np

from concourse import bacc, bass, bass_isa, mybir, tile
from concourse import bass_utils

F32 = mybir.dt.float32
P = 128
C = 256
M = 196
B = 256
NCORES = 8
S = B // NCORES            # samples per core
NTRIU = C * (C + 1) // 2   # 32896

# matmul input dtype for the big products
MM_DT = mybir.dt.float16

LAST_EXEC_NS = None
LAST_RESULTS = None


def build(tc, y_ap, x_ap, ident_ap, icons_ap, n_samples=S):
    nc = tc.nc
    import contextlib

    with contextlib.ExitStack() as ctx:
        consts = ctx.enter_context(tc.tile_pool(name="consts", bufs=1))
        fpool = ctx.enter_context(tc.tile_pool(name="fpool", bufs=1))
        work = ctx.enter_context(tc.tile_pool(name="work", bufs=2))
        mats = ctx.enter_context(tc.tile_pool(name="mats", bufs=2))
        psum = ctx.enter_context(tc.tile_pool(name="psum", bufs=8, space="PSUM"))

        ident = consts.tile([P, P], MM_DT, tag="ident")
        nc.sync.dma_start(ident[:], ident_ap[:])
        icons = consts.tile([P, 3, 2 * C], MM_DT, tag="icons")
        nc.sync.dma_start(icons[:], icons_ap[:])

        # Per 8-sample group: [P, 8, 384] — cols 0:256 = F rows 0:128 (full),
        # cols 256:384 = F22 (rows 128:256, cols 128:256). F21 = F12^T on host.
        GRP = 8
        ngrp = (n_samples + GRP - 1) // GRP
        ftg = [
            fpool.tile([P, GRP, 384], F32, tag=f"ft{g}", name=f"ft{g}")
            for g in range(ngrp)
        ]
        flushed = [False] * ngrp
        _dbg_x = None

        def prod(U, V):
            """One [128,512] PSUM bank <- U @ V (both [P,512] fp16, symmetric)."""
            p_t = psum.tile([P, 2 * C], F32, tag="ps_big")
            for mt in range(2):
                oc = slice(mt * C, (mt + 1) * C)
                ms0 = slice(mt * P, mt * P + P)
                ms1 = slice(C + mt * P, C + mt * P + P)
                nc.tensor.matmul(
                    p_t[:, oc], U[:, ms0], V[:, 0:C], start=True, stop=False
                )
                nc.tensor.matmul(
                    p_t[:, oc], U[:, ms1], V[:, C : 2 * C], start=False, stop=True
                )
            return p_t

        def prod_f(U, V):
            """Final product: skip F21 (host reconstructs it from F12^T).
            cols 0:256 = F rows 0:128 full; cols 384:512 = F22."""
            p_t = psum.tile([P, 2 * C], F32, tag="ps_big")
            nc.tensor.matmul(p_t[:, 0:C], U[:, 0:P], V[:, 0:C], start=True, stop=False)
            nc.tensor.matmul(
                p_t[:, 0:C], U[:, C : C + P], V[:, C : 2 * C], start=False, stop=True
            )
            nc.tensor.matmul(
                p_t[:, C + P : 2 * C], U[:, P:C], V[:, P:C], start=True, stop=False
            )
            nc.tensor.matmul(
                p_t[:, C + P : 2 * C], U[:, C + P : 2 * C], V[:, C + P : 2 * C],
                start=False, stop=True,
            )
            return p_t

        def sample_stages(b):
            """Yield closures for one sample's pipeline stages; tiles tagged
            by b%2 so a pair of samples uses disjoint pool slots and their
            PE bursts interleave (keeps the PE dense enough to stay warm)."""
            x = {}
            nonlocal _dbg_x
            _dbg_x = x
            fx = f"_{b % 3}"

            def load():
                x["xr"] = work.tile([P, 2, M], F32, tag="xr" + fx, name="xr" + fx)
                nc.sync.dma_start(
                    x["xr"][:], x_ap[b].rearrange("(h p) m -> p h m", p=P)
                )

            def stats():
                xr = x["xr"]
                mean2 = work.tile([P, 2], F32, tag="mean2" + fx, name="mean2" + fx)
                nc.vector.tensor_reduce(
                    mean2[:], xr[:], axis=mybir.AxisListType.X,
                    op=mybir.AluOpType.add,
                )
                negmean = work.tile([P, 2], F32, tag="negmean" + fx, name="nm" + fx)
                nc.vector.tensor_scalar_mul(negmean[:], mean2[:], -1.0 / M)
                xc = work.tile([P, 2, M], MM_DT, tag="xc" + fx, name="xc" + fx)
                for h in range(2):
                    nc.scalar.activation(
                        xc[:, h], xr[:, h],
                        mybir.ActivationFunctionType.Identity,
                        bias=negmean[:, h : h + 1],
                    )
                # sum(xc^2): square on DVE (fp16 2x rate), reduce per half
                sq = work.tile([P, 2, M], MM_DT, tag="sq" + fx, name="sq" + fx)
                nc.vector.tensor_mul(sq[:], xc[:], xc[:])
                s2 = work.tile([P, 2], F32, tag="s2" + fx, name="s2" + fx)
                nc.vector.tensor_reduce(
                    s2[:], sq[:], axis=mybir.AxisListType.X,
                    op=mybir.AluOpType.add,
                )
                x["xc"], x["s2"] = xc, s2

            def trace():
                s2r = work.tile([P, 2], F32, tag="s2r" + fx, name="s2r" + fx)
                nc.gpsimd.partition_all_reduce(
                    s2r[:], x["s2"][:], channels=P, reduce_op=bass_isa.ReduceOp.add
                )
                trv = work.tile([P, 1], F32, tag="trv" + fx, name="trv" + fx)
                nc.vector.tensor_tensor(
                    trv[:], s2r[:, 0:1], s2r[:, 1:2], op=mybir.AluOpType.add
                )
                inv = work.tile([P, 1], F32, tag="inv" + fx, name="inv" + fx)
                abv2 = work.tile([P, 1], F32, tag="abv2" + fx, name="abv2" + fx)
                nc.vector.reciprocal(inv[:], trv[:])
                nc.scalar.activation(
                    abv2[:], trv[:], mybir.ActivationFunctionType.Sqrt,
                    scale=1.0 / (M * 8192.0 * 8192.0),
                )
                x["inv"], x["abv2"] = inv, abv2

            def transpose():
                xc = x["xc"]
                tp = psum.tile([P, 2 * C], MM_DT, tag="ps_big", name="tp" + fx)
                for h in range(2):
                    nc.tensor.transpose(
                        tp[:, h * P : h * P + P], xc[:, h, 0:P], ident[:]
                    )
                    nc.tensor.transpose(
                        tp[0 : M - P, C + h * P : C + h * P + P], xc[:, h, P:M],
                        ident[:],
                    )
                x["tp"] = tp

            def scale_xcT():
                tp = x["tp"]
                xcT0 = work.tile([P, C], MM_DT, tag="xcT0" + fx, name="xcT0" + fx)
                xcT1 = work.tile([P, C], MM_DT, tag="xcT1" + fx, name="xcT1" + fx)
                nc.scalar.activation(
                    xcT0[:], tp[:, 0:C], mybir.ActivationFunctionType.Copy
                )
                nc.vector.tensor_copy(xcT1[0 : M - P], tp[0 : M - P, C : 2 * C])
                x["xcT0"], x["xcT1"] = xcT0, xcT1

            def gram():
                xcT0, xcT1 = x["xcT0"], x["xcT1"]
                a_ps = psum.tile([P, 2 * C], F32, tag="ps_big", name="aps" + fx)
                for mt in range(2):
                    oc = slice(mt * C, (mt + 1) * C)
                    ms = slice(mt * P, (mt + 1) * P)
                    nc.tensor.matmul(
                        a_ps[:, oc], xcT0[:, ms], xcT0[:], start=True, stop=False
                    )
                    nc.tensor.matmul(
                        a_ps[:, oc], xcT1[0 : M - P, ms], xcT1[0 : M - P, :],
                        start=False, stop=True,
                    )
                x["a_ps"] = a_ps

            def mat(tag):
                t = mats.tile([P, 2 * C], MM_DT, tag=tag + fx, name=tag + fx)
                x[tag] = t
                return t

            def drain_A():
                # A_s = (M*cov)/trace(M*cov): fold 1/tr into the PSUM drain
                nc.scalar.activation(
                    mat("A")[:], x["a_ps"][:], mybir.ActivationFunctionType.Copy,
                    scale=x["inv"][:, 0:1],
                )

            def zy1():
                nc.vector.tensor_tensor(
                    mat("ZY1")[:], icons[:, 0, :], x["A"][:],
                    op=mybir.AluOpType.subtract,
                )

            def mk_prod(dst, u, v):
                def f():
                    x[dst] = prod(x[u], x[v])
                return f

            def mk_prod_f():
                x["f_ps"] = prod_f(x["Y2"], x["ZY3"])

            def drain(dst, src, eng):
                def f():
                    t = mat(dst)
                    if eng == "act":
                        nc.scalar.activation(
                            t[:], x[src][:], mybir.ActivationFunctionType.Copy
                        )
                    else:
                        nc.vector.tensor_copy(t[:], x[src][:])
                return f

            def sub(dst, k, src):
                def f():
                    nc.vector.tensor_tensor(
                        mat(dst)[:], icons[:, k, :], x[src][:],
                        op=mybir.AluOpType.subtract,
                    )
                return f

            def fstore():
                f_ps, abv2 = x["f_ps"], x["abv2"]
                ft = ftg[b // GRP]
                bi = b % GRP
                nc.vector.tensor_scalar_mul(
                    ft[:, bi, 0:C], f_ps[:, 0:C], abv2[:, 0:1]
                )
                nc.scalar.activation(
                    ft[:, bi, C : C + P], f_ps[:, C + P : 2 * C],
                    mybir.ActivationFunctionType.Copy, scale=abv2[:, 0:1],
                )

            return [
                load, stats, trace, transpose, scale_xcT, gram,
                drain_A, zy1,
                mk_prod("y1_ps", "A", "ZY1"), drain("Y1", "y1_ps", "act"),
                mk_prod("w1_ps", "ZY1", "Y1"), sub("ZY2", 1, "w1_ps"),
                mk_prod("y2_ps", "Y1", "ZY2"), drain("Y2", "y2_ps", "dve"),
                mk_prod("z2_ps", "ZY2", "ZY1"), drain("Z2", "z2_ps", "act"),
                mk_prod("w2_ps", "Z2", "Y2"), sub("ZY3", 2, "w2_ps"),
                mk_prod_f, fstore,
            ]

        if n_samples == 1:
            sstg = sample_stages(0)
            for st in sstg:
                st()
            dbg = {}
            for nm_ in ["A", "ZY1", "Y1", "ZY2", "Y2", "Z2", "ZY3"]:
                d_ap = nc.dram_tensor(f"dbg_{nm_}", (P, 2 * C), F32,
                                      kind="ExternalOutput").ap()
                t32 = work.tile([P, 2 * C], F32, tag=f"dbg{nm_}")
                nc.vector.tensor_copy(t32[:], _dbg_x[nm_][:])
                nc.sync.dma_start(d_ap[:], t32[:])
            for nm_ in ["inv", "abv2", "s2"]:
                d_ap = nc.dram_tensor(f"dbg_{nm_}", (P, 1), F32,
                                      kind="ExternalOutput").ap()
                nc.sync.dma_start(d_ap[:], _dbg_x[nm_][:, 0:1])
            for nm_, rr in [("xcT0", P), ("xcT1", M - P)]:
                d_ap = nc.dram_tensor(f"dbg_{nm_}", (P, C), F32,
                                      kind="ExternalOutput").ap()
                t32 = work.tile([P, C], F32, tag=f"dbg{nm_}")
                nc.vector.tensor_copy(t32[0:rr], _dbg_x[nm_][0:rr])
                nc.sync.dma_start(d_ap[0:rr], t32[0:rr])
            return

        for b0 in range(0, n_samples, 3):
            grp = [sample_stages(b) for b in range(b0, min(b0 + 3, n_samples))]
            n = len(grp[0])
            for step in range(n + 2):
                for i, sg in enumerate(grp):
                    if 0 <= step - i < n:
                        sg[step - i]()
            # flush any output group fully produced by now (overlaps compute)
            done = min(b0 + 3, n_samples)
            for g in range(ngrp):
                if g * GRP + GRP <= done and not flushed[g]:
                    nc.sync.dma_start(
                        y_ap[g * GRP : (g + 1) * GRP].rearrange("s p c -> p s c"),
                        ftg[g][:],
                    )
                    flushed[g] = True
        for g in range(ngrp):  # tail flush (partial last group)
            if not flushed[g]:
                w = min(n_samples - g * GRP, GRP)
                nc.sync.dma_start(
                    y_ap[g * GRP : g * GRP + w].rearrange("s p c -> p s c"),
                    ftg[g][:, 0:w],
                )
                flushed[g] = True


def _make_const_inputs():
    # icons[:, k, :]: [3I, 12I, 768I] in concatenated row-tile layout:
    # cols 0:256 = matrix rows 0:128 (diag at col p),
    # cols 256:512 = matrix rows 128:256 (diag at col 256+128+p).
    e = np.zeros((P, 2 * C), np.float32)
    e[np.arange(P), np.arange(P)] = 1.0
    e[np.arange(P), C + P + np.arange(P)] = 1.0
    icons = np.stack([3.0 * e, 12.0 * e, 768.0 * e], axis=1).astype(np.float16)
    return {
        "ident": np.eye(P, dtype=np.float16),
        "icons": np.ascontiguousarray(icons),
    }


def make_nc(n_samples=S, num_devices=NCORES):
    nc = bacc.Bacc(
        "TRN2",
        target_bir_lowering=False,
        debug=False,
        enable_asserts=False,
        num_devices=num_devices,
    )
    x_ap = nc.dram_tensor("x", (n_samples, C, M), F32, kind="ExternalInput").ap()
    y_ap = nc.dram_tensor("y", (n_samples, P, 384), F32, kind="ExternalOutput").ap()
    ident_ap = nc.dram_tensor("ident", (P, P), MM_DT, kind="ExternalInput").ap()
    icons_ap = nc.dram_tensor("icons", (P, 3, 2 * C), MM_DT, kind="ExternalInput").ap()
    with tile.TileContext(nc) as tc:
        build(tc, y_ap, x_ap, ident_ap, icons_ap, n_samples)
    nc.compile()
    return nc


def kernel(x, _trace=False, **_trace_kwargs):
    global LAST_EXEC_NS, LAST_RESULTS
    x = np.ascontiguousarray(np.asarray(x), dtype=np.float32)
    assert x.shape == (B, C, 14, 14)
    xr = x.reshape(B, C, M)

    nc = make_nc()
    consts = _make_const_inputs()
    in_maps = [
        {"x": np.ascontiguousarray(xr[i * S : (i + 1) * S]), **consts}
        for i in range(NCORES)
    ]
    res = bass_utils.run_bass_kernel_spmd(
        nc, in_maps, core_ids=list(range(NCORES)), trace=_trace, **_trace_kwargs
    )
    LAST_EXEC_NS = res.exec_time_ns
    LAST_RESULTS = res
    yd = np.concatenate([r["y"] for r in res.results], axis=0)  # [B, 128, 384]
    full = np.empty((B, C, C), np.float32)
    full[:, 0:P, :] = yd[:, :, 0:C]                       # F rows 0:128
    full[:, P:C, P:C] = yd[:, :, C : C + P]               # F22
    full[:, P:C, 0:P] = yd[:, :, P:C].transpose(0, 2, 1)  # F21 = F12^T
    i, j = np.triu_indices(C)
    return np.ascontiguousarray(full.reshape(B, C * C)[:, i * C + j])

